# revision 23
# baseline (speedup 1.0000x reference)
"""Multi-head attention (qkv pointwise-conv projection + softmax attention)
on 8 Trainium2 NeuronCores.

Problem shapes (hardcoded):
    x:     [B=4, D=512, L=2048] f32
    w_qkv: [3*D=1536, D=512]    f32
    out:   [B, D, L]            f32

Sharding: 2 cores per batch element; each core owns 4 of the 8 heads
(tensor-parallel on the qkv output channels). Core c -> batch c//2,
head group c%2 (heads 4*(c%2) .. 4*(c%2)+3).

Per-core kernel (all in bf16 compute, f32 accumulate):
    Q/K proj:  q[o,l] = sum_d w[o,d] x[d,l]   (layout [head_dim, L])
    V proj  :  vT[l,o]                          (layout [L, head_dim])
               vT stored per head with a fused ones-column -> attn@[v|1]
               yields both the weighted values and the softmax denominator.
    scores  :  St[j,i] = sum_d k[d,j] q[d,i]  (two heads packed in the
               128-row PE array via row tiling: head0 partitions 0-63,
               head1 partitions 64-127 -- the two matmuls run concurrently)
    softmax :  exp on ScalarE (scale folded into the activation), no max
               subtraction (scores are O(1) by construction)
    attn@v  :  O[d(+den),i] accumulated over j blocks in PSUM
    norm    :  O[d,i] * broadcast(1/den[i])

The kernel is ScalarE-bound: 16.8M exps/core at ~1 elem/cycle/lane is
~130us of ACTIVATE.  Everything else (PE ~106us ideal, DVE ~60us, DMA
~15us) is scheduled to hide under the exp stream:
  - input DMA split across the two HWDGE rings (sync + scalar) so the
    first scores land ~4us earlier
  - per-block epilogues (den -> 1/den -> broadcast -> mul -> DMA) are
    emitted inside the NEXT block's early slots so their DVE work never
    delays the projection CASTs that feed upcoming score matmuls
  - projections all complete by block 2 so late block boundaries are
    dependency-free
  - the final block's epilogue is split across ScalarE+VectorE with a
    bf16 PE broadcast to shorten the kernel tail
"""

import os
import numpy as np

B, D, L, H = 4, 512, 2048, 8
HD = D // H  # 64
N_CORES = 8
SCALE = float(D) ** -0.5

# module-level knobs for test.py; harness uses defaults
TRACE = False
LAST_RESULTS = None

_COMPILED = {}


def _build_nc():
    from contextlib import ExitStack

    import concourse.bass as bass
    import concourse.mybir as mybir
    import concourse.tile as tile
    from concourse.bacc import Bacc

    F32 = mybir.dt.float32
    BF16 = mybir.dt.bfloat16
    Exp = mybir.ActivationFunctionType.Exp

    # Bacc (not plain Bass): its finalize() runs the legalization passes that
    # split multi-wait matmuls (walrus MM struct supports only 1 sync wait).
    nc = Bacc("TRN2", target_bir_lowering=False, debug=False)
    # host pre-permuted layouts -> fully contiguous DMA descriptors (4-6KB)
    # x: [p, lc, dc, l'] where d = dc*128+p, l = lc*512+l'
    x_d = nc.dram_tensor("x", [128, 4, 4, 512], BF16, kind="ExternalInput")
    # wT split per head-pair (q|k interleaved): [p, pair, dc, o'] where
    # o' 0:128 = q cols of the pair, 128:256 = k cols -- so the first
    # projection only needs the 256KB pair-0 slice, not all of wqk
    wqk_d = nc.dram_tensor("wqkT", [128, 2, 4, 256], BF16, kind="ExternalInput")
    wv_d = nc.dram_tensor("wvT", [128, 4, 256], BF16, kind="ExternalInput")
    out_d = nc.dram_tensor("out", [256, L], F32, kind="ExternalOutput")

    NJB = L // 128  # 16 key blocks
    NIC = L // 512  # 4 query chunks

    with ExitStack() as ctx:
        tc = ctx.enter_context(tile.TileContext(nc))
        const = ctx.enter_context(tc.tile_pool(name="const", bufs=1))
        qkp = ctx.enter_context(tc.tile_pool(name="qkp", bufs=1))
        vtp = ctx.enter_context(tc.tile_pool(name="vtp", bufs=1))
        sx = ctx.enter_context(tc.tile_pool(name="sx", bufs=6))
        nrm = ctx.enter_context(tc.tile_pool(name="nrm", bufs=4))
        outp = ctx.enter_context(tc.tile_pool(name="outp", bufs=4))
        drp = ctx.enter_context(tc.tile_pool(name="drp", bufs=4, space="DRAM"))
        ps_st = ctx.enter_context(tc.tile_pool(name="ps_st", bufs=2, space="PSUM"))
        ps_o = ctx.enter_context(tc.tile_pool(name="ps_o", bufs=4, space="PSUM"))

        # ---- PE warmup + load inputs ----
        # a few matmuls on zeros keep the PE busy through the input-DMA window
        # so the HAM clock gate opens (1.2 -> 2.4 GHz) before real work.
        scr_sb = const.tile([128, 512], BF16, tag="scr")
        nc.vector.memset(scr_sb[:], 0.0)
        warm_ps = ps_st.tile([128, 1024], F32, tag="st", name="warm")
        for _ in range(8):
            nc.tensor.matmul(warm_ps[:, 0:512], scr_sb[:, 0:128], scr_sb[:])
        # Input DMA split across BOTH HWDGE rings (sync + scalar) so the
        # first projection's inputs (wqk halves + x chunk 0 halves) land in
        # parallel.  The scalar ring only carries pre-stream triggers; every
        # mid-stream DMA stays on sync so the exp stream is never interrupted.
        wqk_sb = const.tile([128, 2, 4, 256], BF16, tag="wqk")
        wv_sb = const.tile([128, 4, 256], BF16, tag="wv")
        x_sb = const.tile([128, 4, 4, 512], BF16, tag="x")
        # each HWDGE ring sustains only ~90GB/s here (and the gpsimd SWDGE
        # ring contends with HWDGE on the SBUF ports -- measured net loss),
        # so everything is halved across the sync+scalar rings in need
        # order: pair-0 weights + x0 gate the first scores, then x1 (key
        # block 4), wv (first attn@v, deferrable), x2, x3, pair-1 weights.
        nc.sync.dma_start(out=wqk_sb[:, 0, 0:2, :], in_=wqk_d[:, 0, 0:2, :])
        nc.scalar.dma_start(out=wqk_sb[:, 0, 2:4, :], in_=wqk_d[:, 0, 2:4, :])
        nc.sync.dma_start(out=x_sb[:, 0, 0:2, :], in_=x_d[:, 0, 0:2, :])
        nc.scalar.dma_start(out=x_sb[:, 0, 2:4, :], in_=x_d[:, 0, 2:4, :])
        nc.sync.dma_start(out=x_sb[:, 1, 0:2, :], in_=x_d[:, 1, 0:2, :])
        nc.scalar.dma_start(out=x_sb[:, 1, 2:4, :], in_=x_d[:, 1, 2:4, :])
        nc.scalar.dma_start(out=wv_sb[:], in_=wv_d[:])
        nc.sync.dma_start(out=x_sb[:, 2, 0:2, :], in_=x_d[:, 2, 0:2, :])
        nc.scalar.dma_start(out=x_sb[:, 2, 2:4, :], in_=x_d[:, 2, 2:4, :])
        nc.sync.dma_start(out=x_sb[:, 3, 0:2, :], in_=x_d[:, 3, 0:2, :])
        nc.scalar.dma_start(out=x_sb[:, 3, 2:4, :], in_=x_d[:, 3, 2:4, :])
        nc.scalar.dma_start(out=wqk_sb[:, 1, :, :], in_=wqk_d[:, 1, :, :])
        ones_sb = const.tile([1, 64], F32, tag="ones")
        nc.vector.memset(ones_sb[:], 1.0)
        onesb_sb = const.tile([1, 64], BF16, tag="onesb")
        nc.vector.memset(onesb_sb[:], 1.0)

        q_sb = [qkp.tile([128, L], BF16, tag=f"q{p}", name=f"q{p}") for p in range(2)]
        k_sb = [qkp.tile([128, L], BF16, tag=f"k{p}", name=f"k{p}") for p in range(2)]
        vt_sb = [vtp.tile([128, 4, 65], BF16, tag=f"vt{jb}", name=f"vt{jb}") for jb in range(NJB)]

        # Projection groups run in 1-bank [128,512] PSUM tiles from the shared
        # "o" pool so they never contend with the exp-feeding st pipeline.
        def g_qk(p, sec, lc):
            # one 512-wide column group of the Q (sec=0) or K (sec=256)
            # projection for head-pair p
            def f():
                dst = q_sb[p] if sec == 0 else k_sb[p]
                oo = 0 if sec == 0 else 128
                ps = ps_o.tile([128, 512], F32, tag="o", name="projg")
                for dc in range(4):
                    nc.tensor.matmul(
                        ps[:],
                        wqk_sb[:, p, dc, oo : oo + 128],
                        x_sb[:, lc, dc, :],
                        start=(dc == 0),
                        stop=(dc == 3),
                    )
                nc.vector.tensor_copy(dst[:, lc * 512 : (lc + 1) * 512], ps[:])

            return f

        def g_vt(jb):
            def f():
                nc.vector.memset(vt_sb[jb][:, :, 64:65], 1.0)
                ps = ps_o.tile([128, 512], F32, tag="o", name="projv")
                for dc in range(4):
                    nc.tensor.matmul(
                        ps[:, 0:256],
                        x_sb[:, jb // 4, dc, (jb % 4) * 128 : (jb % 4 + 1) * 128],
                        wv_sb[:, dc, :],
                        start=(dc == 0),
                        stop=(dc == 3),
                    )
                nc.vector.tensor_copy(
                    vt_sb[jb][:, :, 0:64],
                    ps[:, 0:256].rearrange("par (h e) -> par h e", e=64),
                )

            return f

        def attn_block(p, ic, fillers=(), finish_prev=(), last=False, defer_attnv=0):
            # scores+softmax+attn@v for head pair p, query chunk ic (512 wide)
            # fillers: {jb: [callables]} -- projection groups interleaved into
            # the loop to fill PE slack without starving ScalarE
            # finish_prev: the previous block's per-head epilogues, emitted a
            # couple of slots in (so their DVE work queues behind this block's
            # early projection CASTs, not in front of them)
            # Returns this block's epilogue closures.
            fillers = dict(fillers)
            i0 = ic * 512

            def st_mms(jb):
                # St[j, i] for both heads of the pair, row-packed in the PE
                st = ps_st.tile([128, 1024], F32, tag="st")
                for hp in range(2):
                    nc.tensor.matmul(
                        st[:, hp * 512 : (hp + 1) * 512],
                        k_sb[p][hp * 64 : (hp + 1) * 64, jb * 128 : (jb + 1) * 128],
                        q_sb[p][hp * 64 : (hp + 1) * 64, i0 : i0 + 512],
                        start=True,
                        stop=True,
                    )
                return st

            o_ps = [ps_o.tile([65, 512], F32, tag="o", name="o_acc") for _ in range(2)]
            # epilogue emission slots: in filler-carrying blocks the early
            # slots' projection CASTs must hit the DVE queue before the
            # epilogue's ~2us of copies, or the next proj group's PSUM buf
            # rotation stalls the PE FIFO (and with it the score matmuls)
            ep_slots = (2, 4) if not fillers else (3, 7)

            def attnv(jb, se):
                for hp in range(2):
                    nc.tensor.matmul(
                        o_ps[hp][:],
                        vt_sb[jb][:, 2 * p + hp, :],
                        se[:, hp * 512 : (hp + 1) * 512],
                        start=(jb == 0),
                        stop=(jb == NJB - 1),
                    )

            backlog = []
            st_cur = st_mms(0)
            for jb in range(NJB):
                se = sx.tile([128, 1024], BF16, tag="se")
                if last and jb == NJB - 1:
                    # split the final exp by head so head0's attn@v (and the
                    # epilogue chain behind it) starts half a slot earlier
                    nc.scalar.activation(
                        se[:, 0:512], st_cur[:, 0:512], Exp, scale=SCALE
                    )
                    nc.scalar.activation(
                        se[:, 512:1024], st_cur[:, 512:1024], Exp, scale=SCALE
                    )
                else:
                    nc.scalar.activation(se[:], st_cur[:], Exp, scale=SCALE)
                if jb + 1 < NJB:
                    st_cur = st_mms(jb + 1)
                for f in fillers.get(jb, ()):
                    f()
                if jb == ep_slots[0] and len(finish_prev) > 0:
                    finish_prev[0]()
                if jb == ep_slots[1] and len(finish_prev) > 1:
                    finish_prev[1]()
                if jb < defer_attnv:
                    # inputs (vt / wv DMA) for the first attn@v groups land
                    # late; deferring their EMISSION keeps them out of the PE
                    # FIFO so they can't head-of-line-block the score matmuls
                    backlog.append((jb, se))
                    continue
                for bjb, bse in backlog:
                    attnv(bjb, bse)
                backlog = []
                attnv(jb, se)

            def finish_head(hp):
                # normalize and write out: o is copied to SBUF FIRST so its
                # PSUM bank frees immediately (projection groups of the
                # following block rotate through the same pool); then 1/den on
                # DVE (fast approx), row broadcast across 64 partitions via a
                # DRAM bounce (latency hides under the exp stream), multiply.
                def f():
                    hh = 2 * p + hp
                    o = o_ps[hp]
                    den_sb = nrm.tile([1, 512], F32, tag="den")
                    nc.vector.tensor_copy(den_sb[:], o[64:65, :])
                    osb = outp.tile([64, 512], F32, tag="osb")
                    nc.vector.tensor_copy(osb[:], o[0:64, :])
                    recip = nrm.tile([1, 512], F32, tag="recip")
                    # NB: approx-recip needs SBUF input at matching partition
                    # offset -- feeding it o[64:65] (partition 64) directly
                    # returns garbage; bounce through a partition-0 tile first
                    nc.vector.reciprocal_approx_fast(out=recip[:], in_=den_sb[:])
                    rbc = nrm.tile([64, 512], F32, tag="rbc")
                    dbounce = drp.tile([1, 512], F32, tag="db", name="db")
                    nc.sync.dma_start(out=dbounce[:], in_=recip[:])
                    nc.sync.dma_start(
                        out=rbc[:],
                        in_=bass.AP(
                            tensor=dbounce.tensor,
                            offset=dbounce.offset,
                            ap=[[0, 64], [1, 512]],
                        ),
                    )
                    ot = outp.tile([64, 512], F32, tag="ot")
                    nc.vector.tensor_mul(ot[:], osb[:], rbc[:])
                    nc.sync.dma_start(
                        out=out_d[hh * 64 : (hh + 1) * 64, i0 : i0 + 512], in_=ot[:]
                    )

                return f

            def finish_last():
                # Kernel-tail epilogue: spread the chain across ScalarE (idle
                # now) and VectorE, broadcast 1/den with a cheap bf16 PE
                # matmul, and split the two output DMAs across both HWDGE
                # rings.  ~5us instead of ~7.5us of serial DVE.
                o0, o1 = o_ps
                den0 = nrm.tile([1, 512], F32, tag="den")
                nc.vector.tensor_copy(den0[:], o0[64:65, :])
                den1 = nrm.tile([1, 512], F32, tag="den")
                nc.scalar.copy(den1[:], o1[64:65, :])
                r0 = nrm.tile([1, 512], F32, tag="recip")
                nc.vector.reciprocal_approx_fast(out=r0[:], in_=den0[:])
                rb0 = nrm.tile([1, 512], BF16, tag="rb16")
                nc.scalar.copy(rb0[:], r0[:])
                r1 = nrm.tile([1, 512], F32, tag="recip")
                nc.vector.reciprocal_approx_fast(out=r1[:], in_=den1[:])
                rb1 = nrm.tile([1, 512], BF16, tag="rb16")
                nc.scalar.copy(rb1[:], r1[:])
                bc0 = ps_o.tile([128, 512], F32, tag="o", name="bcast")
                nc.tensor.matmul(bc0[0:64, :], onesb_sb[:], rb0[:], start=True, stop=True)
                bc1 = ps_o.tile([128, 512], F32, tag="o", name="bcast")
                nc.tensor.matmul(bc1[0:64, :], onesb_sb[:], rb1[:], start=True, stop=True)
                rbc0 = nrm.tile([64, 512], F32, tag="rbc")
                nc.scalar.copy(rbc0[:], bc0[0:64, :])
                ot0 = outp.tile([64, 512], F32, tag="ot")
                nc.vector.tensor_mul(ot0[:], o0[0:64, :], rbc0[:])
                hh = 2 * p
                nc.sync.dma_start(
                    out=out_d[hh * 64 : (hh + 1) * 64, i0 : i0 + 512], in_=ot0[:]
                )
                rbc1 = nrm.tile([64, 512], F32, tag="rbc")
                nc.vector.tensor_copy(rbc1[:], bc1[0:64, :])
                ot1 = outp.tile([64, 512], F32, tag="ot")
                nc.vector.tensor_mul(ot1[:], o1[0:64, :], rbc1[:])
                nc.scalar.dma_start(
                    out=out_d[(hh + 1) * 64 : (hh + 2) * 64, i0 : i0 + 512], in_=ot1[:]
                )

            if last:
                return (finish_last,)
            return (finish_head(0), finish_head(1))

        # prologue: just enough projection for the first scores; everything
        # else (vt just-in-time, remaining q0/k0 columns, all of q1/k1) is
        # interleaved so ScalarE starts exp-ing ~12us in and never starves.
        # Constraints: vt[j] before vals(j) of block (0,0); k0 column group m
        # before st(4m); q0 group lc before block (0,lc); q1/k1 before (1,0).
        for f in (g_qk(0, 0, 0), g_qk(0, 256, 0)):
            f()
        fin = ()
        fin = attn_block(0, 0, {
            1: [g_qk(0, 256, 1)],
            3: [g_vt(0), g_vt(1)],
            4: [g_vt(2), g_vt(3)],
            5: [g_qk(0, 256, 2), g_vt(4)],
            6: [g_vt(5), g_vt(6)],
            7: [g_vt(7)],
            8: [g_qk(0, 256, 3), g_vt(8)],
            9: [g_vt(9), g_vt(10)],
            10: [g_qk(0, 0, 1), g_vt(11)],
            11: [g_vt(12)],
            12: [g_vt(13)],
            13: [g_vt(14)],
            14: [g_vt(15)],
        }, finish_prev=fin, defer_attnv=4)
        fin = attn_block(0, 1, {
            0: [g_qk(0, 0, 2)],
            1: [g_qk(0, 0, 3)],
            5: [g_qk(1, 256, 0)],
            6: [g_qk(1, 256, 1)],
            9: [g_qk(1, 256, 2)],
            10: [g_qk(1, 256, 3)],
        }, finish_prev=fin)
        fin = attn_block(0, 2, {
            0: [g_qk(1, 0, 0)],
            1: [g_qk(1, 0, 1)],
            5: [g_qk(1, 0, 2)],
            6: [g_qk(1, 0, 3)],
        }, finish_prev=fin)
        fin = attn_block(0, 3, finish_prev=fin)
        for ic in range(NIC):
            fin = attn_block(1, ic, finish_prev=fin, last=(ic == 3))
        fin[0]()  # final block's epilogue runs inline at the tail

    nc.finalize()
    return nc


def _get_nc():
    if "nc" not in _COMPILED:
        _COMPILED["nc"] = _build_nc()
    return _COMPILED["nc"]


def _prep_inputs(x, w_qkv):
    """Per-core input maps (host-side sharding)."""
    import ml_dtypes

    bf16 = ml_dtypes.bfloat16
    in_maps = []
    for c in range(N_CORES):
        b, g = c // 2, c % 2
        # x[b] [512, 2048] -> [p, lc, dc, l'] so every DMA descriptor is a
        # 4KB contiguous run
        xb = np.ascontiguousarray(
            x[b].reshape(4, 128, 4, 512).transpose(1, 2, 0, 3)
        ).astype(bf16)
        # w rows for this head group, transposed then laid out [p, dc, o];
        # q|k and v as separate tensors so both DMAs are fully contiguous
        wqk_rows = np.concatenate(
            [
                w_qkv[256 * g : 256 * (g + 1), :],
                w_qkv[512 + 256 * g : 512 + 256 * (g + 1), :],
            ],
            axis=0,
        )  # [512, 512]  (q rows 0:256, k rows 256:512)
        wv_rows = w_qkv[1024 + 256 * g : 1024 + 256 * (g + 1), :]  # [256, 512]
        # [part, dc, o] -> [part, pair, dc, 256] with o' = q-half | k-half
        wqkT_flat = np.ascontiguousarray(
            wqk_rows.T.reshape(4, 128, 512).transpose(1, 0, 2)
        )  # [128, 4, 512]
        wqkT = np.empty((128, 2, 4, 256), dtype=np.float32)
        for pr in range(2):
            wqkT[:, pr, :, 0:128] = wqkT_flat[:, :, 128 * pr : 128 * (pr + 1)]
            wqkT[:, pr, :, 128:256] = wqkT_flat[:, :, 256 + 128 * pr : 256 + 128 * (pr + 1)]
        wqkT = np.ascontiguousarray(wqkT).astype(bf16)
        wvT = np.ascontiguousarray(
            wv_rows.T.reshape(4, 128, 256).transpose(1, 0, 2)
        ).astype(bf16)
        in_maps.append({"x": xb, "wqkT": wqkT, "wvT": wvT})
    return in_maps


def kernel(x, w_qkv):
    global LAST_RESULTS
    from concourse.bass_utils import run_bass_kernel_spmd

    nc = _get_nc()
    in_maps = _prep_inputs(np.asarray(x), np.asarray(w_qkv))
    res = run_bass_kernel_spmd(
        nc, in_maps, core_ids=list(range(N_CORES)), trace=TRACE
    )
    LAST_RESULTS = res
    out = np.empty((B, D, L), dtype=np.float32)
    for c in range(N_CORES):
        b, g = c // 2, c % 2
        out[b, 256 * g : 256 * (g + 1), :] = res.results[c]["out"]
    return out


# revision 24
# speedup vs baseline: 1.1765x; 1.1765x over previous
"""Multi-head attention (qkv pointwise-conv projection + softmax attention)
on 8 Trainium2 NeuronCores.

Problem shapes (hardcoded):
    x:     [B=4, D=512, L=2048] f32
    w_qkv: [3*D=1536, D=512]    f32
    out:   [B, D, L]            f32

Sharding: 2 cores per batch element; each core owns 4 of the 8 heads
(tensor-parallel on the qkv output channels). Core c -> batch c//2,
head group c%2 (heads 4*(c%2) .. 4*(c%2)+3).

Per-core kernel (all in bf16 compute, f32 accumulate):
    Q/K proj:  q[o,l] = sum_d w[o,d] x[d,l]   (layout [head_dim, L])
    V proj  :  vT[l,o]                          (layout [L, head_dim])
               vT stored per head with a fused ones-column -> attn@[v|1]
               yields both the weighted values and the softmax denominator.
    scores  :  St[j,i] = sum_d k[d,j] q[d,i]  (two heads packed in the
               128-row PE array via row tiling: head0 partitions 0-63,
               head1 partitions 64-127 -- the two matmuls run concurrently)
    softmax :  exp on ScalarE (scale folded into the activation), no max
               subtraction (scores are O(1) by construction)
    attn@v  :  O[d(+den),i] accumulated over j blocks in PSUM
    norm    :  O[d,i] * broadcast(1/den[i])

The kernel is ScalarE-bound: 16.8M exps/core at ~1 elem/cycle/lane is
~130us of ACTIVATE.  Everything else (PE ~106us ideal, DVE ~60us, DMA
~15us) is scheduled to hide under the exp stream:
  - input DMA split across the two HWDGE rings (sync + scalar) so the
    first scores land ~4us earlier
  - per-block epilogues (den -> 1/den -> broadcast -> mul -> DMA) are
    emitted inside the NEXT block's early slots so their DVE work never
    delays the projection CASTs that feed upcoming score matmuls
  - projections all complete by block 2 so late block boundaries are
    dependency-free
  - the final block's epilogue is split across ScalarE+VectorE with a
    bf16 PE broadcast to shorten the kernel tail
"""

import os
import numpy as np

B, D, L, H = 4, 512, 2048, 8
HD = D // H  # 64
N_CORES = 8
SCALE = float(D) ** -0.5

# module-level knobs for test.py; harness uses defaults
TRACE = False
LAST_RESULTS = None

_COMPILED = {}


def _build_nc():
    from contextlib import ExitStack

    import concourse.bass as bass
    import concourse.mybir as mybir
    import concourse.tile as tile
    from concourse.bacc import Bacc

    F32 = mybir.dt.float32
    BF16 = mybir.dt.bfloat16
    Exp = mybir.ActivationFunctionType.Exp

    # Bacc (not plain Bass): its finalize() runs the legalization passes that
    # split multi-wait matmuls (walrus MM struct supports only 1 sync wait).
    nc = Bacc("TRN2", target_bir_lowering=False, debug=False)
    # host pre-permuted layouts -> fully contiguous DMA descriptors (4-6KB)
    # x: [p, lc, dc, l'] where d = dc*128+p, l = lc*512+l'
    x_d = nc.dram_tensor("x", [128, 4, 4, 512], BF16, kind="ExternalInput")
    # wT split per head-pair (q|k interleaved): [p, pair, dc, o'] where
    # o' 0:128 = q cols of the pair, 128:256 = k cols -- so the first
    # projection only needs the 256KB pair-0 slice, not all of wqk
    wqk_d = nc.dram_tensor("wqkT", [128, 2, 4, 256], BF16, kind="ExternalInput")
    wv_d = nc.dram_tensor("wvT", [128, 4, 256], BF16, kind="ExternalInput")
    out_d = nc.dram_tensor("out", [256, L], F32, kind="ExternalOutput")

    NJB = L // 128  # 16 key blocks
    NIC = L // 512  # 4 query chunks

    with ExitStack() as ctx:
        tc = ctx.enter_context(tile.TileContext(nc))
        const = ctx.enter_context(tc.tile_pool(name="const", bufs=1))
        qkp = ctx.enter_context(tc.tile_pool(name="qkp", bufs=1))
        vtp = ctx.enter_context(tc.tile_pool(name="vtp", bufs=1))
        sx = ctx.enter_context(tc.tile_pool(name="sx", bufs=8))
        nrm = ctx.enter_context(tc.tile_pool(name="nrm", bufs=4))
        outp = ctx.enter_context(tc.tile_pool(name="outp", bufs=4))
        drp = ctx.enter_context(tc.tile_pool(name="drp", bufs=4, space="DRAM"))
        ps_st = ctx.enter_context(tc.tile_pool(name="ps_st", bufs=2, space="PSUM"))
        ps_o = ctx.enter_context(tc.tile_pool(name="ps_o", bufs=4, space="PSUM"))

        # ---- PE warmup + load inputs ----
        # a few matmuls on zeros keep the PE busy through the input-DMA window
        # so the HAM clock gate opens (1.2 -> 2.4 GHz) before real work.
        scr_sb = const.tile([128, 512], BF16, tag="scr")
        nc.vector.memset(scr_sb[:], 0.0)
        warm_ps = ps_st.tile([128, 1024], F32, tag="st", name="warm")
        for _ in range(6):
            nc.tensor.matmul(warm_ps[:, 0:512], scr_sb[:, 0:128], scr_sb[:])
        # Input DMA split across BOTH HWDGE rings (sync + scalar) so the
        # first projection's inputs (wqk halves + x chunk 0 halves) land in
        # parallel.  The scalar ring only carries pre-stream triggers; every
        # mid-stream DMA stays on sync so the exp stream is never interrupted.
        wqk_sb = const.tile([128, 2, 4, 256], BF16, tag="wqk")
        wv_sb = const.tile([128, 4, 256], BF16, tag="wv")
        x_sb = const.tile([128, 4, 4, 512], BF16, tag="x")
        # each HWDGE ring sustains only ~90GB/s here (and the gpsimd SWDGE
        # ring contends with HWDGE on the SBUF ports -- measured net loss),
        # so everything is halved across the sync+scalar rings in need
        # order: pair-0 weights + x0 gate the first scores, then x1 (key
        # block 4), wv (first attn@v, deferrable), x2, x3, pair-1 weights.
        nc.sync.dma_start(out=wqk_sb[:, 0, 0:2, :], in_=wqk_d[:, 0, 0:2, :])
        nc.scalar.dma_start(out=wqk_sb[:, 0, 2:4, :], in_=wqk_d[:, 0, 2:4, :])
        nc.sync.dma_start(out=x_sb[:, 0, 0:2, :], in_=x_d[:, 0, 0:2, :])
        nc.scalar.dma_start(out=x_sb[:, 0, 2:4, :], in_=x_d[:, 0, 2:4, :])
        nc.sync.dma_start(out=x_sb[:, 1, 0:2, :], in_=x_d[:, 1, 0:2, :])
        nc.scalar.dma_start(out=x_sb[:, 1, 2:4, :], in_=x_d[:, 1, 2:4, :])
        nc.scalar.dma_start(out=wv_sb[:], in_=wv_d[:])
        nc.sync.dma_start(out=x_sb[:, 2, 0:2, :], in_=x_d[:, 2, 0:2, :])
        nc.scalar.dma_start(out=x_sb[:, 2, 2:4, :], in_=x_d[:, 2, 2:4, :])
        nc.sync.dma_start(out=x_sb[:, 3, 0:2, :], in_=x_d[:, 3, 0:2, :])
        nc.scalar.dma_start(out=x_sb[:, 3, 2:4, :], in_=x_d[:, 3, 2:4, :])
        nc.scalar.dma_start(out=wqk_sb[:, 1, :, :], in_=wqk_d[:, 1, :, :])
        ones_sb = const.tile([1, 64], F32, tag="ones")
        nc.vector.memset(ones_sb[:], 1.0)
        onesb_sb = const.tile([1, 64], BF16, tag="onesb")
        nc.vector.memset(onesb_sb[:], 1.0)

        q_sb = [qkp.tile([128, L], BF16, tag=f"q{p}", name=f"q{p}") for p in range(2)]
        k_sb = [qkp.tile([128, L], BF16, tag=f"k{p}", name=f"k{p}") for p in range(2)]
        vt_sb = [vtp.tile([128, 4, 65], BF16, tag=f"vt{jb}", name=f"vt{jb}") for jb in range(NJB)]

        # Projection groups run in 1-bank [128,512] PSUM tiles from the shared
        # "o" pool so they never contend with the exp-feeding st pipeline.
        def g_qk(p, sec, lc):
            # one 512-wide column group of the Q (sec=0) or K (sec=256)
            # projection for head-pair p
            def f():
                dst = q_sb[p] if sec == 0 else k_sb[p]
                oo = 0 if sec == 0 else 128
                ps = ps_o.tile([128, 512], F32, tag="o", name="projg")
                for dc in range(4):
                    nc.tensor.matmul(
                        ps[:],
                        wqk_sb[:, p, dc, oo : oo + 128],
                        x_sb[:, lc, dc, :],
                        start=(dc == 0),
                        stop=(dc == 3),
                    )
                nc.vector.tensor_copy(dst[:, lc * 512 : (lc + 1) * 512], ps[:])

            return f

        def g_vt(jb):
            def f():
                nc.vector.memset(vt_sb[jb][:, :, 64:65], 1.0)
                ps = ps_o.tile([128, 512], F32, tag="o", name="projv")
                for dc in range(4):
                    nc.tensor.matmul(
                        ps[:, 0:256],
                        x_sb[:, jb // 4, dc, (jb % 4) * 128 : (jb % 4 + 1) * 128],
                        wv_sb[:, dc, :],
                        start=(dc == 0),
                        stop=(dc == 3),
                    )
                nc.vector.tensor_copy(
                    vt_sb[jb][:, :, 0:64],
                    ps[:, 0:256].rearrange("par (h e) -> par h e", e=64),
                )

            return f

        def attn_block(p, ic, fillers=(), finish_prev=(), last=False, defer_attnv=0):
            # scores+softmax+attn@v for head pair p, query chunk ic (512 wide)
            # fillers: {jb: [callables]} -- projection groups interleaved into
            # the loop to fill PE slack without starving ScalarE
            # finish_prev: the previous block's per-head epilogues, emitted a
            # couple of slots in (so their DVE work queues behind this block's
            # early projection CASTs, not in front of them)
            # Returns this block's epilogue closures.
            fillers = dict(fillers)
            i0 = ic * 512

            def st_mms(jb):
                # St[j, i] for both heads of the pair, row-packed in the PE
                st = ps_st.tile([128, 1024], F32, tag="st")
                for hp in range(2):
                    nc.tensor.matmul(
                        st[:, hp * 512 : (hp + 1) * 512],
                        k_sb[p][hp * 64 : (hp + 1) * 64, jb * 128 : (jb + 1) * 128],
                        q_sb[p][hp * 64 : (hp + 1) * 64, i0 : i0 + 512],
                        start=True,
                        stop=True,
                    )
                return st

            o_ps = [ps_o.tile([65, 512], F32, tag="o", name="o_acc") for _ in range(2)]
            # epilogue emission slots: in filler-carrying blocks the early
            # slots' projection CASTs must hit the DVE queue before the
            # epilogue's ~2us of copies, or the next proj group's PSUM buf
            # rotation stalls the PE FIFO (and with it the score matmuls)
            ep_slots = (2, 4) if not fillers else (3, 7)

            def attnv(jb, se):
                for hp in range(2):
                    nc.tensor.matmul(
                        o_ps[hp][:],
                        vt_sb[jb][:, 2 * p + hp, :],
                        se[:, hp * 512 : (hp + 1) * 512],
                        start=(jb == 0),
                        stop=(jb == NJB - 1),
                    )

            backlog = []
            st_cur = st_mms(0)
            for jb in range(NJB):
                se = sx.tile([128, 1024], BF16, tag="se")
                if last and jb == NJB - 1:
                    # split the final exp by head so head0's attn@v (and the
                    # epilogue chain behind it) starts half a slot earlier
                    nc.scalar.activation(
                        se[:, 0:512], st_cur[:, 0:512], Exp, scale=SCALE
                    )
                    nc.scalar.activation(
                        se[:, 512:1024], st_cur[:, 512:1024], Exp, scale=SCALE
                    )
                else:
                    nc.scalar.activation(se[:], st_cur[:], Exp, scale=SCALE)
                if jb + 1 < NJB:
                    st_cur = st_mms(jb + 1)
                for f in fillers.get(jb, ()):
                    f()
                if jb == ep_slots[0] and len(finish_prev) > 0:
                    finish_prev[0]()
                if jb == ep_slots[1] and len(finish_prev) > 1:
                    finish_prev[1]()
                if jb < defer_attnv:
                    # inputs (vt / wv DMA) for the first attn@v groups land
                    # late; deferring their EMISSION keeps them out of the PE
                    # FIFO so they can't head-of-line-block the score matmuls
                    backlog.append((jb, se))
                    continue
                for bjb, bse in backlog:
                    attnv(bjb, bse)
                backlog = []
                attnv(jb, se)

            def finish_head(hp):
                # normalize and write out: o is copied to SBUF FIRST so its
                # PSUM bank frees immediately (projection groups of the
                # following block rotate through the same pool); then 1/den on
                # DVE (fast approx), row broadcast across 64 partitions via a
                # DRAM bounce (latency hides under the exp stream), multiply.
                def f():
                    hh = 2 * p + hp
                    o = o_ps[hp]
                    den_sb = nrm.tile([1, 512], F32, tag="den")
                    nc.vector.tensor_copy(den_sb[:], o[64:65, :])
                    osb = outp.tile([64, 512], F32, tag="osb")
                    nc.vector.tensor_copy(osb[:], o[0:64, :])
                    recip = nrm.tile([1, 512], F32, tag="recip")
                    # NB: approx-recip needs SBUF input at matching partition
                    # offset -- feeding it o[64:65] (partition 64) directly
                    # returns garbage; bounce through a partition-0 tile first
                    nc.vector.reciprocal_approx_fast(out=recip[:], in_=den_sb[:])
                    rbc = nrm.tile([64, 512], F32, tag="rbc")
                    dbounce = drp.tile([1, 512], F32, tag="db", name="db")
                    nc.sync.dma_start(out=dbounce[:], in_=recip[:])
                    nc.sync.dma_start(
                        out=rbc[:],
                        in_=bass.AP(
                            tensor=dbounce.tensor,
                            offset=dbounce.offset,
                            ap=[[0, 64], [1, 512]],
                        ),
                    )
                    ot = outp.tile([64, 512], F32, tag="ot")
                    nc.vector.tensor_mul(ot[:], osb[:], rbc[:])
                    nc.sync.dma_start(
                        out=out_d[hh * 64 : (hh + 1) * 64, i0 : i0 + 512], in_=ot[:]
                    )

                return f

            def finish_last():
                # Kernel-tail epilogue: spread the chain across ScalarE (idle
                # now) and VectorE, broadcast 1/den with a cheap bf16 PE
                # matmul, and split the two output DMAs across both HWDGE
                # rings.  ~5us instead of ~7.5us of serial DVE.
                o0, o1 = o_ps
                den0 = nrm.tile([1, 512], F32, tag="den")
                nc.vector.tensor_copy(den0[:], o0[64:65, :])
                den1 = nrm.tile([1, 512], F32, tag="den")
                nc.scalar.copy(den1[:], o1[64:65, :])
                r0 = nrm.tile([1, 512], F32, tag="recip")
                nc.vector.reciprocal_approx_fast(out=r0[:], in_=den0[:])
                rb0 = nrm.tile([1, 512], BF16, tag="rb16")
                nc.scalar.copy(rb0[:], r0[:])
                r1 = nrm.tile([1, 512], F32, tag="recip")
                nc.vector.reciprocal_approx_fast(out=r1[:], in_=den1[:])
                rb1 = nrm.tile([1, 512], BF16, tag="rb16")
                nc.scalar.copy(rb1[:], r1[:])
                bc0 = ps_o.tile([128, 512], F32, tag="o", name="bcast")
                nc.tensor.matmul(bc0[0:64, :], onesb_sb[:], rb0[:], start=True, stop=True)
                bc1 = ps_o.tile([128, 512], F32, tag="o", name="bcast")
                nc.tensor.matmul(bc1[0:64, :], onesb_sb[:], rb1[:], start=True, stop=True)
                rbc0 = nrm.tile([64, 512], F32, tag="rbc")
                nc.scalar.copy(rbc0[:], bc0[0:64, :])
                ot0 = outp.tile([64, 512], F32, tag="ot")
                nc.vector.tensor_mul(ot0[:], o0[0:64, :], rbc0[:])
                hh = 2 * p
                nc.sync.dma_start(
                    out=out_d[hh * 64 : (hh + 1) * 64, i0 : i0 + 512], in_=ot0[:]
                )
                rbc1 = nrm.tile([64, 512], F32, tag="rbc")
                nc.vector.tensor_copy(rbc1[:], bc1[0:64, :])
                ot1 = outp.tile([64, 512], F32, tag="ot")
                nc.vector.tensor_mul(ot1[:], o1[0:64, :], rbc1[:])
                nc.scalar.dma_start(
                    out=out_d[(hh + 1) * 64 : (hh + 2) * 64, i0 : i0 + 512], in_=ot1[:]
                )

            if last:
                return (finish_last,)
            return (finish_head(0), finish_head(1))

        # prologue: just enough projection for the first scores; everything
        # else (vt just-in-time, remaining q0/k0 columns, all of q1/k1) is
        # interleaved so ScalarE starts exp-ing ~12us in and never starves.
        # Constraints: vt[j] before vals(j) of block (0,0); k0 column group m
        # before st(4m); q0 group lc before block (0,lc); q1/k1 before (1,0).
        for f in (g_qk(0, 0, 0), g_qk(0, 256, 0)):
            f()
        fin = ()
        fin = attn_block(0, 0, {
            1: [g_qk(0, 256, 1)],
            3: [g_vt(0), g_vt(1)],
            4: [g_vt(2), g_vt(3)],
            5: [g_qk(0, 256, 2), g_vt(4)],
            6: [g_vt(5), g_vt(6)],
            7: [g_vt(7)],
            8: [g_qk(0, 256, 3), g_vt(8)],
            9: [g_vt(9), g_vt(10)],
            10: [g_qk(0, 0, 1), g_vt(11)],
            11: [g_vt(12)],
            12: [g_vt(13)],
            13: [g_vt(14)],
            14: [g_vt(15)],
        }, finish_prev=fin, defer_attnv=4)
        fin = attn_block(0, 1, {
            0: [g_qk(0, 0, 2)],
            1: [g_qk(0, 0, 3)],
            5: [g_qk(1, 256, 0)],
            6: [g_qk(1, 256, 1)],
            9: [g_qk(1, 256, 2)],
            10: [g_qk(1, 256, 3)],
        }, finish_prev=fin)
        fin = attn_block(0, 2, {
            0: [g_qk(1, 0, 0)],
            1: [g_qk(1, 0, 1)],
            5: [g_qk(1, 0, 2)],
            6: [g_qk(1, 0, 3)],
        }, finish_prev=fin)
        fin = attn_block(0, 3, finish_prev=fin)
        for ic in range(NIC):
            fin = attn_block(1, ic, finish_prev=fin, last=(ic == 3))
        fin[0]()  # final block's epilogue runs inline at the tail

    nc.finalize()
    return nc


def _get_nc():
    if "nc" not in _COMPILED:
        _COMPILED["nc"] = _build_nc()
    return _COMPILED["nc"]


def _prep_inputs(x, w_qkv):
    """Per-core input maps (host-side sharding)."""
    import ml_dtypes

    bf16 = ml_dtypes.bfloat16
    in_maps = []
    for c in range(N_CORES):
        b, g = c // 2, c % 2
        # x[b] [512, 2048] -> [p, lc, dc, l'] so every DMA descriptor is a
        # 4KB contiguous run
        xb = np.ascontiguousarray(
            x[b].reshape(4, 128, 4, 512).transpose(1, 2, 0, 3)
        ).astype(bf16)
        # w rows for this head group, transposed then laid out [p, dc, o];
        # q|k and v as separate tensors so both DMAs are fully contiguous
        wqk_rows = np.concatenate(
            [
                w_qkv[256 * g : 256 * (g + 1), :],
                w_qkv[512 + 256 * g : 512 + 256 * (g + 1), :],
            ],
            axis=0,
        )  # [512, 512]  (q rows 0:256, k rows 256:512)
        wv_rows = w_qkv[1024 + 256 * g : 1024 + 256 * (g + 1), :]  # [256, 512]
        # [part, dc, o] -> [part, pair, dc, 256] with o' = q-half | k-half
        wqkT_flat = np.ascontiguousarray(
            wqk_rows.T.reshape(4, 128, 512).transpose(1, 0, 2)
        )  # [128, 4, 512]
        wqkT = np.empty((128, 2, 4, 256), dtype=np.float32)
        for pr in range(2):
            wqkT[:, pr, :, 0:128] = wqkT_flat[:, :, 128 * pr : 128 * (pr + 1)]
            wqkT[:, pr, :, 128:256] = wqkT_flat[:, :, 256 + 128 * pr : 256 + 128 * (pr + 1)]
        wqkT = np.ascontiguousarray(wqkT).astype(bf16)
        wvT = np.ascontiguousarray(
            wv_rows.T.reshape(4, 128, 256).transpose(1, 0, 2)
        ).astype(bf16)
        in_maps.append({"x": xb, "wqkT": wqkT, "wvT": wvT})
    return in_maps


def kernel(x, w_qkv):
    global LAST_RESULTS
    from concourse.bass_utils import run_bass_kernel_spmd

    nc = _get_nc()
    in_maps = _prep_inputs(np.asarray(x), np.asarray(w_qkv))
    res = run_bass_kernel_spmd(
        nc, in_maps, core_ids=list(range(N_CORES)), trace=TRACE
    )
    LAST_RESULTS = res
    out = np.empty((B, D, L), dtype=np.float32)
    for c in range(N_CORES):
        b, g = c // 2, c % 2
        out[b, 256 * g : 256 * (g + 1), :] = res.results[c]["out"]
    return out


# revision 27
# speedup vs baseline: 1.2134x; 1.0313x over previous
"""Multi-head attention (qkv pointwise-conv projection + softmax attention)
on 8 Trainium2 NeuronCores.

Problem shapes (hardcoded):
    x:     [B=4, D=512, L=2048] f32
    w_qkv: [3*D=1536, D=512]    f32
    out:   [B, D, L]            f32

Sharding: 2 cores per batch element; each core owns 4 of the 8 heads
(tensor-parallel on the qkv output channels). Core c -> batch c//2,
head group c%2 (heads 4*(c%2) .. 4*(c%2)+3).

Per-core kernel (all in bf16 compute, f32 accumulate):
    Q/K proj:  q[o,l] = sum_d w[o,d] x[d,l]   (layout [head_dim, L])
    V proj  :  vT[l,o]                          (layout [L, head_dim])
               vT stored per head with a fused ones-column -> attn@[v|1]
               yields both the weighted values and the softmax denominator.
    scores  :  St[j,i] = sum_d k[d,j] q[d,i]  (two heads packed in the
               128-row PE array via row tiling: head0 partitions 0-63,
               head1 partitions 64-127 -- the two matmuls run concurrently)
    softmax :  exp on ScalarE (scale folded into the activation), no max
               subtraction (scores are O(1) by construction)
    attn@v  :  O[d(+den),i] accumulated over j blocks in PSUM
    norm    :  O[d,i] * broadcast(1/den[i])

The kernel is ScalarE-bound: 16.8M exps/core at ~1 elem/cycle/lane is
~130us of ACTIVATE.  Everything else (PE ~106us ideal, DVE ~60us, DMA
~15us) is scheduled to hide under the exp stream:
  - input DMA split across the two HWDGE rings (sync + scalar) so the
    first scores land ~4us earlier
  - per-block epilogues (den -> 1/den -> broadcast -> mul -> DMA) are
    emitted inside the NEXT block's early slots so their DVE work never
    delays the projection CASTs that feed upcoming score matmuls
  - projections all complete by block 2 so late block boundaries are
    dependency-free
  - the final block's epilogue is split across ScalarE+VectorE with a
    bf16 PE broadcast to shorten the kernel tail
"""

import os
import numpy as np

B, D, L, H = 4, 512, 2048, 8
HD = D // H  # 64
N_CORES = 8
SCALE = float(D) ** -0.5

# module-level knobs for test.py; harness uses defaults
TRACE = False
LAST_RESULTS = None

_COMPILED = {}


def _build_nc():
    from contextlib import ExitStack

    import concourse.bass as bass
    import concourse.mybir as mybir
    import concourse.tile as tile
    from concourse.bacc import Bacc

    F32 = mybir.dt.float32
    BF16 = mybir.dt.bfloat16
    Exp = mybir.ActivationFunctionType.Exp

    # Bacc (not plain Bass): its finalize() runs the legalization passes that
    # split multi-wait matmuls (walrus MM struct supports only 1 sync wait).
    nc = Bacc("TRN2", target_bir_lowering=False, debug=False)
    # host pre-permuted layouts -> fully contiguous DMA descriptors (4-6KB)
    # x: [p, lc, dc, l'] where d = dc*128+p, l = lc*512+l'
    x_d = nc.dram_tensor("x", [128, 4, 4, 512], BF16, kind="ExternalInput")
    # wT split per head-pair (q|k interleaved): [p, pair, dc, o'] where
    # o' 0:128 = q cols of the pair, 128:256 = k cols -- so the first
    # projection only needs the 256KB pair-0 slice, not all of wqk
    wqk_d = nc.dram_tensor("wqkT", [128, 2, 4, 256], BF16, kind="ExternalInput")
    wv_d = nc.dram_tensor("wvT", [128, 4, 256], BF16, kind="ExternalInput")
    out_d = nc.dram_tensor("out", [256, L], F32, kind="ExternalOutput")

    NJB = L // 128  # 16 key blocks
    NIC = L // 512  # 4 query chunks

    with ExitStack() as ctx:
        tc = ctx.enter_context(tile.TileContext(nc))
        const = ctx.enter_context(tc.tile_pool(name="const", bufs=1))
        qkp = ctx.enter_context(tc.tile_pool(name="qkp", bufs=1))
        vtp = ctx.enter_context(tc.tile_pool(name="vtp", bufs=1))
        sx = ctx.enter_context(tc.tile_pool(name="sx", bufs=8))
        nrm = ctx.enter_context(tc.tile_pool(name="nrm", bufs=4))
        outp = ctx.enter_context(tc.tile_pool(name="outp", bufs=4))
        drp = ctx.enter_context(tc.tile_pool(name="drp", bufs=4, space="DRAM"))
        ps_st = ctx.enter_context(tc.tile_pool(name="ps_st", bufs=2, space="PSUM"))
        ps_o = ctx.enter_context(tc.tile_pool(name="ps_o", bufs=4, space="PSUM"))

        # ---- PE warmup + load inputs ----
        # a few matmuls on zeros keep the PE busy through the input-DMA window
        # so the HAM clock gate opens (1.2 -> 2.4 GHz) before real work.
        scr_sb = const.tile([128, 512], BF16, tag="scr")
        nc.vector.memset(scr_sb[:], 0.0)
        warm_ps = ps_st.tile([128, 1024], F32, tag="st", name="warm")
        for _ in range(8):
            nc.tensor.matmul(warm_ps[:, 0:512], scr_sb[:, 0:128], scr_sb[:])
        # Input DMA split across BOTH HWDGE rings (sync + scalar) so the
        # first projection's inputs (wqk halves + x chunk 0 halves) land in
        # parallel.  The scalar ring only carries pre-stream triggers; every
        # mid-stream DMA stays on sync so the exp stream is never interrupted.
        wqk_sb = const.tile([128, 2, 4, 256], BF16, tag="wqk")
        wv_sb = const.tile([128, 4, 256], BF16, tag="wv")
        x_sb = const.tile([128, 4, 4, 512], BF16, tag="x")
        # each HWDGE ring sustains only ~90GB/s here (and the gpsimd SWDGE
        # ring contends with HWDGE on the SBUF ports -- measured net loss),
        # so everything is halved across the sync+scalar rings in need
        # order: pair-0 weights + x0 gate the first scores, then x1 (key
        # block 4), wv (first attn@v, deferrable), x2, x3, pair-1 weights.
        nc.sync.dma_start(out=wqk_sb[:, 0, 0:2, :], in_=wqk_d[:, 0, 0:2, :])
        nc.scalar.dma_start(out=wqk_sb[:, 0, 2:4, :], in_=wqk_d[:, 0, 2:4, :])
        nc.sync.dma_start(out=x_sb[:, 0, 0:2, :], in_=x_d[:, 0, 0:2, :])
        nc.scalar.dma_start(out=x_sb[:, 0, 2:4, :], in_=x_d[:, 0, 2:4, :])
        nc.sync.dma_start(out=x_sb[:, 1, 0:2, :], in_=x_d[:, 1, 0:2, :])
        nc.scalar.dma_start(out=x_sb[:, 1, 2:4, :], in_=x_d[:, 1, 2:4, :])
        nc.scalar.dma_start(out=wv_sb[:], in_=wv_d[:])
        nc.sync.dma_start(out=x_sb[:, 2, 0:2, :], in_=x_d[:, 2, 0:2, :])
        nc.scalar.dma_start(out=x_sb[:, 2, 2:4, :], in_=x_d[:, 2, 2:4, :])
        nc.sync.dma_start(out=x_sb[:, 3, 0:2, :], in_=x_d[:, 3, 0:2, :])
        nc.scalar.dma_start(out=x_sb[:, 3, 2:4, :], in_=x_d[:, 3, 2:4, :])
        nc.scalar.dma_start(out=wqk_sb[:, 1, :, :], in_=wqk_d[:, 1, :, :])
        ones_sb = const.tile([1, 64], F32, tag="ones")
        nc.vector.memset(ones_sb[:], 1.0)
        onesb_sb = const.tile([1, 64], BF16, tag="onesb")
        nc.vector.memset(onesb_sb[:], 1.0)

        q_sb = [qkp.tile([128, L], BF16, tag=f"q{p}", name=f"q{p}") for p in range(2)]
        k_sb = [qkp.tile([128, L], BF16, tag=f"k{p}", name=f"k{p}") for p in range(2)]
        vt_sb = [vtp.tile([128, 4, 65], BF16, tag=f"vt{jb}", name=f"vt{jb}") for jb in range(NJB)]

        # Projection groups run in 1-bank [128,512] PSUM tiles from the shared
        # "o" pool so they never contend with the exp-feeding st pipeline.
        def g_qk(p, sec, lc):
            # one 512-wide column group of the Q (sec=0) or K (sec=256)
            # projection for head-pair p
            def f():
                dst = q_sb[p] if sec == 0 else k_sb[p]
                oo = 0 if sec == 0 else 128
                ps = ps_o.tile([128, 512], F32, tag="pj", bufs=2, name="projg")
                for dc in range(4):
                    nc.tensor.matmul(
                        ps[:],
                        wqk_sb[:, p, dc, oo : oo + 128],
                        x_sb[:, lc, dc, :],
                        start=(dc == 0),
                        stop=(dc == 3),
                    )
                nc.vector.tensor_copy(dst[:, lc * 512 : (lc + 1) * 512], ps[:])

            return f

        def g_vt(jb):
            def f():
                nc.vector.memset(vt_sb[jb][:, :, 64:65], 1.0)
                ps = ps_o.tile([128, 512], F32, tag="pj", bufs=2, name="projv")
                for dc in range(4):
                    nc.tensor.matmul(
                        ps[:, 0:256],
                        x_sb[:, jb // 4, dc, (jb % 4) * 128 : (jb % 4 + 1) * 128],
                        wv_sb[:, dc, :],
                        start=(dc == 0),
                        stop=(dc == 3),
                    )
                nc.vector.tensor_copy(
                    vt_sb[jb][:, :, 0:64],
                    ps[:, 0:256].rearrange("par (h e) -> par h e", e=64),
                )

            return f

        def attn_block(p, ic, fillers=(), finish_prev=(), last=False, defer_attnv=0):
            # scores+softmax+attn@v for head pair p, query chunk ic (512 wide)
            # fillers: {jb: [callables]} -- projection groups interleaved into
            # the loop to fill PE slack without starving ScalarE
            # finish_prev: the previous block's per-head epilogues, emitted a
            # couple of slots in (so their DVE work queues behind this block's
            # early projection CASTs, not in front of them)
            # Returns this block's epilogue closures.
            fillers = dict(fillers)
            i0 = ic * 512

            def st_mms(jb):
                # St[j, i] for both heads of the pair, row-packed in the PE
                st = ps_st.tile([128, 1024], F32, tag="st")
                for hp in range(2):
                    nc.tensor.matmul(
                        st[:, hp * 512 : (hp + 1) * 512],
                        k_sb[p][hp * 64 : (hp + 1) * 64, jb * 128 : (jb + 1) * 128],
                        q_sb[p][hp * 64 : (hp + 1) * 64, i0 : i0 + 512],
                        start=True,
                        stop=True,
                    )
                return st

            o_ps = [ps_o.tile([65, 512], F32, tag="o", bufs=2, name="o_acc") for _ in range(2)]
            # epilogue emission slots: in filler-carrying blocks the early
            # slots' projection CASTs must hit the DVE queue before the
            # epilogue's ~2us of copies, or the next proj group's PSUM buf
            # rotation stalls the PE FIFO (and with it the score matmuls)
            ep_slots = (2, 4) if not fillers else (3, 7)

            def attnv(jb, hp, se):
                nc.tensor.matmul(
                    o_ps[hp][:],
                    vt_sb[jb][:, 2 * p + hp, :],
                    se[:, hp * 512 : (hp + 1) * 512],
                    start=(jb == 0),
                    stop=(jb == NJB - 1),
                )

            # Per-head attn@v EMISSION deferral: head hp's o_acc buffer is only
            # freed by the previous block's finish_head(hp), emitted at
            # ep_slots[hp] -- an attn@v emitted before that would sit in the PE
            # FIFO waiting for the buffer and head-of-line-block the score
            # matmuls behind it.  Block 0 instead waits for late vt/wv inputs.
            d_h = (
                (4, 5)
                if defer_attnv
                else (ep_slots[0] + 1, ep_slots[1] + 1) if finish_prev else (0, 0)
            )
            backlog = ([], [])
            st_cur = st_mms(0)
            for jb in range(NJB):
                se = sx.tile([128, 1024], BF16, tag="se")
                if last and jb == NJB - 1:
                    # split the final exp by head so head0's attn@v (and the
                    # epilogue chain behind it) starts half a slot earlier
                    nc.scalar.activation(
                        se[:, 0:512], st_cur[:, 0:512], Exp, scale=SCALE
                    )
                    nc.scalar.activation(
                        se[:, 512:1024], st_cur[:, 512:1024], Exp, scale=SCALE
                    )
                else:
                    nc.scalar.activation(se[:], st_cur[:], Exp, scale=SCALE)
                if jb + 1 < NJB:
                    st_cur = st_mms(jb + 1)
                for f in fillers.get(jb, ()):
                    f()
                if jb == ep_slots[0] and len(finish_prev) > 0:
                    finish_prev[0]()
                if jb == ep_slots[1] and len(finish_prev) > 1:
                    finish_prev[1]()
                for hp in range(2):
                    if jb < d_h[hp]:
                        backlog[hp].append((jb, se))
                        continue
                    for bjb, bse in backlog[hp]:
                        attnv(bjb, hp, bse)
                    del backlog[hp][:]
                    attnv(jb, hp, se)

            def finish_head(hp):
                # normalize and write out: o is copied to SBUF FIRST so its
                # PSUM bank frees immediately (projection groups of the
                # following block rotate through the same pool); then 1/den on
                # DVE (fast approx), row broadcast across 64 partitions via a
                # DRAM bounce (latency hides under the exp stream), multiply.
                def f():
                    hh = 2 * p + hp
                    o = o_ps[hp]
                    den_sb = nrm.tile([1, 512], F32, tag="den")
                    nc.vector.tensor_copy(den_sb[:], o[64:65, :])
                    osb = outp.tile([64, 512], F32, tag="osb")
                    nc.vector.tensor_copy(osb[:], o[0:64, :])
                    recip = nrm.tile([1, 512], F32, tag="recip")
                    # NB: approx-recip needs SBUF input at matching partition
                    # offset -- feeding it o[64:65] (partition 64) directly
                    # returns garbage; bounce through a partition-0 tile first
                    nc.vector.reciprocal_approx_fast(out=recip[:], in_=den_sb[:])
                    rbc = nrm.tile([64, 512], F32, tag="rbc")
                    dbounce = drp.tile([1, 512], F32, tag="db", name="db")
                    nc.sync.dma_start(out=dbounce[:], in_=recip[:])
                    nc.sync.dma_start(
                        out=rbc[:],
                        in_=bass.AP(
                            tensor=dbounce.tensor,
                            offset=dbounce.offset,
                            ap=[[0, 64], [1, 512]],
                        ),
                    )
                    ot = outp.tile([64, 512], F32, tag="ot")
                    nc.vector.tensor_mul(ot[:], osb[:], rbc[:])
                    nc.sync.dma_start(
                        out=out_d[hh * 64 : (hh + 1) * 64, i0 : i0 + 512], in_=ot[:]
                    )

                return f

            def finish_last():
                # Kernel-tail epilogue: spread the chain across ScalarE (idle
                # now) and VectorE, broadcast 1/den with a cheap bf16 PE
                # matmul, and split the two output DMAs across both HWDGE
                # rings.  ~5us instead of ~7.5us of serial DVE.
                o0, o1 = o_ps
                den0 = nrm.tile([1, 512], F32, tag="den")
                nc.vector.tensor_copy(den0[:], o0[64:65, :])
                den1 = nrm.tile([1, 512], F32, tag="den")
                nc.scalar.copy(den1[:], o1[64:65, :])
                r0 = nrm.tile([1, 512], F32, tag="recip")
                nc.vector.reciprocal_approx_fast(out=r0[:], in_=den0[:])
                rb0 = nrm.tile([1, 512], BF16, tag="rb16")
                nc.scalar.copy(rb0[:], r0[:])
                r1 = nrm.tile([1, 512], F32, tag="recip")
                nc.vector.reciprocal_approx_fast(out=r1[:], in_=den1[:])
                rb1 = nrm.tile([1, 512], BF16, tag="rb16")
                nc.scalar.copy(rb1[:], r1[:])
                bc0 = ps_o.tile([128, 512], F32, tag="pj", bufs=2, name="bcast")
                nc.tensor.matmul(bc0[0:64, :], onesb_sb[:], rb0[:], start=True, stop=True)
                bc1 = ps_o.tile([128, 512], F32, tag="pj", bufs=2, name="bcast")
                nc.tensor.matmul(bc1[0:64, :], onesb_sb[:], rb1[:], start=True, stop=True)
                rbc0 = nrm.tile([64, 512], F32, tag="rbc")
                nc.scalar.copy(rbc0[:], bc0[0:64, :])
                ot0 = outp.tile([64, 512], F32, tag="ot")
                nc.vector.tensor_mul(ot0[:], o0[0:64, :], rbc0[:])
                hh = 2 * p
                nc.sync.dma_start(
                    out=out_d[hh * 64 : (hh + 1) * 64, i0 : i0 + 512], in_=ot0[:]
                )
                rbc1 = nrm.tile([64, 512], F32, tag="rbc")
                nc.vector.tensor_copy(rbc1[:], bc1[0:64, :])
                ot1 = outp.tile([64, 512], F32, tag="ot")
                nc.vector.tensor_mul(ot1[:], o1[0:64, :], rbc1[:])
                nc.scalar.dma_start(
                    out=out_d[(hh + 1) * 64 : (hh + 2) * 64, i0 : i0 + 512], in_=ot1[:]
                )

            if last:
                return (finish_last,)
            return (finish_head(0), finish_head(1))

        # prologue: just enough projection for the first scores; everything
        # else (vt just-in-time, remaining q0/k0 columns, all of q1/k1) is
        # interleaved so ScalarE starts exp-ing ~12us in and never starves.
        # Constraints: vt[j] before vals(j) of block (0,0); k0 column group m
        # before st(4m); q0 group lc before block (0,lc); q1/k1 before (1,0).
        for f in (g_qk(0, 0, 0), g_qk(0, 256, 0)):
            f()
        fin = ()
        fin = attn_block(0, 0, {
            1: [g_qk(0, 256, 1)],
            3: [g_vt(0), g_vt(1)],
            4: [g_vt(2), g_vt(3)],
            5: [g_qk(0, 256, 2), g_vt(4)],
            6: [g_vt(5), g_vt(6)],
            7: [g_vt(7)],
            8: [g_qk(0, 256, 3), g_vt(8)],
            9: [g_vt(9), g_vt(10)],
            10: [g_qk(0, 0, 1), g_vt(11)],
            11: [g_vt(12)],
            12: [g_vt(13)],
            13: [g_vt(14)],
            14: [g_vt(15)],
        }, finish_prev=fin, defer_attnv=4)
        fin = attn_block(0, 1, {
            0: [g_qk(0, 0, 2)],
            1: [g_qk(0, 0, 3)],
            5: [g_qk(1, 256, 0)],
            6: [g_qk(1, 256, 1)],
            9: [g_qk(1, 256, 2)],
            10: [g_qk(1, 256, 3)],
        }, finish_prev=fin)
        fin = attn_block(0, 2, {
            0: [g_qk(1, 0, 0)],
            1: [g_qk(1, 0, 1)],
            5: [g_qk(1, 0, 2)],
            6: [g_qk(1, 0, 3)],
        }, finish_prev=fin)
        fin = attn_block(0, 3, finish_prev=fin)
        for ic in range(NIC):
            fin = attn_block(1, ic, finish_prev=fin, last=(ic == 3))
        fin[0]()  # final block's epilogue runs inline at the tail

    nc.finalize()
    return nc


def _get_nc():
    if "nc" not in _COMPILED:
        _COMPILED["nc"] = _build_nc()
    return _COMPILED["nc"]


def _prep_inputs(x, w_qkv):
    """Per-core input maps (host-side sharding)."""
    import ml_dtypes

    bf16 = ml_dtypes.bfloat16
    in_maps = []
    for c in range(N_CORES):
        b, g = c // 2, c % 2
        # x[b] [512, 2048] -> [p, lc, dc, l'] so every DMA descriptor is a
        # 4KB contiguous run
        xb = np.ascontiguousarray(
            x[b].reshape(4, 128, 4, 512).transpose(1, 2, 0, 3)
        ).astype(bf16)
        # w rows for this head group, transposed then laid out [p, dc, o];
        # q|k and v as separate tensors so both DMAs are fully contiguous
        wqk_rows = np.concatenate(
            [
                w_qkv[256 * g : 256 * (g + 1), :],
                w_qkv[512 + 256 * g : 512 + 256 * (g + 1), :],
            ],
            axis=0,
        )  # [512, 512]  (q rows 0:256, k rows 256:512)
        wv_rows = w_qkv[1024 + 256 * g : 1024 + 256 * (g + 1), :]  # [256, 512]
        # [part, dc, o] -> [part, pair, dc, 256] with o' = q-half | k-half
        wqkT_flat = np.ascontiguousarray(
            wqk_rows.T.reshape(4, 128, 512).transpose(1, 0, 2)
        )  # [128, 4, 512]
        wqkT = np.empty((128, 2, 4, 256), dtype=np.float32)
        for pr in range(2):
            wqkT[:, pr, :, 0:128] = wqkT_flat[:, :, 128 * pr : 128 * (pr + 1)]
            wqkT[:, pr, :, 128:256] = wqkT_flat[:, :, 256 + 128 * pr : 256 + 128 * (pr + 1)]
        wqkT = np.ascontiguousarray(wqkT).astype(bf16)
        wvT = np.ascontiguousarray(
            wv_rows.T.reshape(4, 128, 256).transpose(1, 0, 2)
        ).astype(bf16)
        in_maps.append({"x": xb, "wqkT": wqkT, "wvT": wvT})
    return in_maps


def kernel(x, w_qkv):
    global LAST_RESULTS
    from concourse.bass_utils import run_bass_kernel_spmd

    nc = _get_nc()
    in_maps = _prep_inputs(np.asarray(x), np.asarray(w_qkv))
    res = run_bass_kernel_spmd(
        nc, in_maps, core_ids=list(range(N_CORES)), trace=TRACE
    )
    LAST_RESULTS = res
    out = np.empty((B, D, L), dtype=np.float32)
    for c in range(N_CORES):
        b, g = c // 2, c % 2
        out[b, 256 * g : 256 * (g + 1), :] = res.results[c]["out"]
    return out
